# revision 13
# baseline (speedup 1.0000x reference)
"""Trainium2 Bass kernel: fused ViT-style attention rollout gating.

Math (per sample b):
  qkT[d, n]   = W_qk[d, :] @ x[b][:, n]          (d = 2*896: q rows then k rows)
  logits[h]   = qT_h.T @ kT_h                    ([49, 49] per head, K = head_dim = 128)
  attn[h]     = softmax(scale * logits[h])       (row-wise; no max-subtraction: |logits*scale| < 2)
  fused       = min_h attn[h]                    ([49, 49])
  rowsum[n]   = sum_m fused[n, m];  colsum[m] = sum_n fused[n, m]
  att[m]      = (colsum[m] + 1) / (49 * (rowsum[m] + 1))
  rx[b]       = x[b] * (1 + att)                 (broadcast over channels)

The reference's flat-topk masking quirk only touches global sample 0; it is
corrected exactly on the host from the device-exported `fused` matrices.

Sharding: pure data-parallel, 128 samples per core across 8 cores.
Layout per core/sub-batch (SB=16 samples): x is loaded as 7 c-tiles
[128c, 16b, 49n]; attention tiles pack 16 samples as 2 vertical partition
blocks (base 0 / 64, via PE column tiling) x 8 horizontal 49-col slots.
"""

import numpy as np
import ml_dtypes

# ---- problem constants (hardcoded per contest rules) ----
B_FULL = 1024
C = 896
N = 49                   # tokens (7x7)
NH = 7                   # heads
HD = 128                 # head dim
NCORES = 8
B_CORE = B_FULL // NCORES   # 128
SB = 16                     # samples per sub-batch
NSB = B_CORE // SB          # 8 sub-batches
CT = C // 128               # 7 contraction tiles
MT = 2 * C // 128           # 14 output d-tiles (q then k)
HF = 8 * N                  # 392 = half free width (8 horizontal samples)
FDX = SB * N                # 784
SCALE = float(HD) ** -0.5
NN = N * N                  # 2401
KEEP = NN - int(NN * 0.9)   # 241 largest kept out of topk(smallest 90%)

_CACHE = {}
LAST_RESULTS = None  # BassKernelResults of the most recent kernel() call


def _build(nsb=NSB):
    import concourse.tile as tile
    from concourse import bacc, mybir

    dt = mybir.dt
    f32 = dt.float32
    bf16 = dt.bfloat16
    AF = mybir.ActivationFunctionType
    ALU = mybir.AluOpType
    AX = mybir.AxisListType

    nc = bacc.Bacc("TRN2", target_bir_lowering=False, debug=False,
                   num_devices=NCORES)
    x_d = nc.dram_tensor("x", [B_CORE, C, N], f32, kind="ExternalInput").ap()
    wt_d = nc.dram_tensor("wt", [C, MT * 128], bf16, kind="ExternalInput").ap()
    rx_d = nc.dram_tensor("rx", [B_CORE, C, N], f32, kind="ExternalOutput").ap()
    fus_d = nc.dram_tensor("fus", [NSB, 2, N, HF], f32,
                           kind="ExternalOutput").ap()

    with tile.TileContext(nc) as tc:
        with (
            tc.tile_pool(name="w", bufs=1) as wpool,
            tc.tile_pool(name="xt", bufs=2) as xtpool,
            tc.tile_pool(name="xb", bufs=2) as xbpool,
            tc.tile_pool(name="qk", bufs=2) as qkpool,
            tc.tile_pool(name="e", bufs=2) as epool,
            tc.tile_pool(name="sm", bufs=2) as spool,
            tc.tile_pool(name="qps", bufs=2, space="PSUM") as qpspool,
            tc.tile_pool(name="aps", bufs=2, space="PSUM") as apspool,
            tc.tile_pool(name="cps", bufs=2, space="PSUM") as cpspool,
            tc.tile_pool(name="dram", bufs=2, space="DRAM") as dpool,
        ):
            # ---- one-time: weights (pre-transposed on host) + colsum ones ----
            wtb = []
            for k in range(CT):
                w = wpool.tile([128, MT * 128], bf16, tag=f"w{k}")
                nc.sync.dma_start(out=w[:], in_=wt_d[128 * k:128 * (k + 1), :])
                wtb.append(w)
            ones2 = wpool.tile([128, 2], f32, tag="ones2")
            nc.vector.memset(ones2[:], 0.0)
            nc.vector.memset(ones2[0:N, 0:1], 1.0)
            nc.vector.memset(ones2[64:64 + N, 1:2], 1.0)

            for s in range(nsb):
                # ---- load x c-tiles + cast to bf16 ----
                xts, xbs = [], []
                for k in range(CT):
                    xt = xtpool.tile([128, SB, N], f32, tag=f"xt{k}")
                    src = x_d[SB * s:SB * (s + 1),
                              128 * k:128 * (k + 1), :].rearrange(
                                  "b c n -> c b n")
                    nc.sync.dma_start(out=xt[:], in_=src)
                    xb = xbpool.tile([128, FDX], bf16, tag=f"xb{k}")
                    nc.vector.tensor_copy(
                        out=xb[:], in_=xt[:].rearrange("c b n -> c (b n)"))
                    xts.append(xt)
                    xbs.append(xb)

                # ---- qkv projection: qkT[m] = W[m-tile] @ x ----
                # qk tiles carry 16 zero columns of tail padding so the
                # M=64-wide attention stationary reads below never run off
                # the end (jj=15 reads columns 735:799)
                qks = []
                for m in range(MT):
                    qk = qkpool.tile([128, FDX + 16], bf16, tag=f"qk{m}")
                    nc.vector.memset(qk[:, FDX:], 0.0)
                    for half in range(2):
                        q = qpspool.tile([128, HF], f32, tag=f"qps{half}")
                        for k in range(CT):
                            nc.tensor.matmul(
                                q[:],
                                lhsT=wtb[k][:, 128 * m:128 * (m + 1)],
                                rhs=xbs[k][:, HF * half:HF * (half + 1)],
                                start=(k == 0), stop=(k == CT - 1))
                        nc.scalar.copy(
                            out=qk[:, HF * half:HF * (half + 1)], in_=q[:])
                    qks.append(qk)

                # ---- attention logits + exp + segmented row sums ----
                S = spool.tile([128, NH, 8], f32, tag="S")
                Es = []
                for h in range(NH):
                    A = apspool.tile([128, HF], f32, tag="A")
                    for j in range(8):
                        # vertical block 0: samples j     -> partitions 0:49
                        # vertical block 1: samples 8 + j -> partitions 64:113
                        # (M=64 stationary: rows 49:64 / 113:128 get junk dot
                        # products from neighboring columns — finite, masked
                        # out downstream — so every PSUM row is written)
                        nc.tensor.matmul(
                            A[0:64, N * j:N * (j + 1)],
                            lhsT=qks[h][:, N * j:N * j + 64],
                            rhs=qks[NH + h][:, N * j:N * (j + 1)],
                            start=True, stop=True)
                        nc.tensor.matmul(
                            A[64:128, N * j:N * (j + 1)],
                            lhsT=qks[h][:, N * (8 + j):N * (8 + j) + 64],
                            rhs=qks[NH + h][:, N * (8 + j):N * (9 + j)],
                            start=True, stop=True)
                    E = epool.tile([128, 8, N], f32, tag=f"E{h}")
                    nc.scalar.activation(
                        out=E[:], in_=A[:].rearrange("p (j n) -> p j n", n=N),
                        func=AF.Exp, scale=SCALE)
                    nc.vector.reduce_sum(out=S[:, h, :], in_=E[:], axis=AX.X)
                    Es.append(E)

                # ---- normalize + min over heads ----
                R = spool.tile([128, NH, 8], f32, tag="R")
                nc.vector.reciprocal(out=R[:], in_=S[:])
                F = spool.tile([128, 8, N], f32, tag="F")
                T = spool.tile([128, 8, N], f32, tag="T")
                for h in range(NH):
                    rb = R[:, h, :].unsqueeze(2).broadcast_to([128, 8, N])
                    dst = F if h == 0 else T
                    nc.vector.tensor_tensor(
                        out=dst[:], in0=Es[h][:], in1=rb, op=ALU.mult)
                    if h > 0:
                        nc.vector.tensor_tensor(
                            out=F[:], in0=F[:], in1=T[:], op=ALU.min)

                # export fused for the host-side topk mask correction
                nc.sync.dma_start(
                    out=fus_d[s, 0],
                    in_=F[0:N].rearrange("p j n -> p (j n)"))
                nc.sync.dma_start(
                    out=fus_d[s, 1],
                    in_=F[64:64 + N].rearrange("p j n -> p (j n)"))

                # ---- rowsum (free-dim reduce) and colsum (PE ones-matmul) ----
                RS = spool.tile([128, 8], f32, tag="RS")
                nc.vector.reduce_sum(out=RS[:], in_=F[:], axis=AX.X)
                Cp = cpspool.tile([2, HF], f32, tag="C")
                nc.tensor.matmul(
                    Cp[:], lhsT=ones2[:],
                    rhs=F[:].rearrange("p j n -> p (j n)"),
                    start=True, stop=True)

                # ---- reshuffle both to [16 samples, 49] via DRAM bounce ----
                rs_dram = dpool.tile([2, 8, N], f32, tag="rsd")
                nc.sync.dma_start(
                    out=rs_dram[0].transpose([1, 0]), in_=RS[0:N, :])
                nc.sync.dma_start(
                    out=rs_dram[1].transpose([1, 0]), in_=RS[64:64 + N, :])
                Rs = spool.tile([SB, N], f32, tag="Rs")
                nc.sync.dma_start(
                    out=Rs[:],
                    in_=rs_dram[:].rearrange("k j n -> (k j) n"))

                Csb = spool.tile([2, 8, N], f32, tag="Csb")
                nc.scalar.copy(
                    out=Csb[:], in_=Cp[:].rearrange("p (j n) -> p j n", n=N))
                cs_dram = dpool.tile([2, 8, N], f32, tag="csd")
                nc.sync.dma_start(out=cs_dram[:], in_=Csb[:])
                Cs = spool.tile([SB, N], f32, tag="Cs")
                nc.sync.dma_start(
                    out=Cs[:],
                    in_=cs_dram[:].rearrange("k j n -> (k j) n"))

                # ---- att + 1 = (colsum+1)/(49*(rowsum+1)) + 1 ----
                D = spool.tile([SB, N], f32, tag="D")
                nc.scalar.activation(out=D[:], in_=Rs[:], func=AF.Copy,
                                     scale=float(N), bias=float(N))
                nc.vector.reciprocal(out=D[:], in_=D[:])
                M1 = spool.tile([SB, N], f32, tag="M1")
                nc.vector.tensor_scalar_add(M1[:], Cs[:], 1.0)
                nc.vector.tensor_tensor(
                    out=M1[:], in0=M1[:], in1=D[:], op=ALU.mult)
                nc.vector.tensor_scalar_add(M1[:], M1[:], 1.0)

                # broadcast multiplier to all 128 partitions via DRAM
                m1_dram = dpool.tile([FDX], f32, tag="m1d")
                nc.sync.dma_start(out=m1_dram[:], in_=M1[:])
                M1b = spool.tile([128, SB, N], f32, tag="M1b")
                nc.sync.dma_start(
                    out=M1b[:],
                    in_=m1_dram[:].rearrange("(b n) -> b n",
                                             n=N).partition_broadcast(128))

                # ---- rx = x * (1 + att), in place, then store ----
                for k in range(CT):
                    nc.vector.tensor_tensor(
                        out=xts[k][:], in0=xts[k][:], in1=M1b[:], op=ALU.mult)
                    dst = rx_d[SB * s:SB * (s + 1),
                               128 * k:128 * (k + 1), :].rearrange(
                                   "b c n -> c b n")
                    nc.sync.dma_start(out=dst, in_=xts[k][:])

    nc.compile()
    return nc


def _get_program(nsb=NSB):
    if nsb not in _CACHE:
        _CACHE[nsb] = _build(nsb)
    return _CACHE[nsb]


def _host_finalize(rx, x5, fused_all):
    """Exact replication of the reference's flat-topk masking quirk.

    Only global sample 0 is affected: its fused matrix is masked by the
    union of all samples' bottom-90% index sets (computed from the
    device-exported fused matrices), then its att row is rebuilt exactly.
    """
    thr = np.partition(fused_all, NN - KEEP, axis=1)[:, NN - KEEP]
    in_top = fused_all >= thr[:, None]
    zero_mask = (~in_top).any(axis=0)
    zero_mask[0] = False
    f0 = fused_all[0].copy()
    f0[zero_mask] = 0.0
    fm = f0.reshape(N, N)
    rowsum = fm.sum(axis=1)
    colsum = fm.sum(axis=0)
    att0 = (colsum + 1.0) / (N * (rowsum + 1.0))
    rx[0] = x5[0] * (1.0 + att0[None, :].astype(np.float32))
    return rx


def kernel(x, W_qkv):
    from concourse.bass_utils import run_bass_kernel_spmd

    nc = _get_program()
    x5 = np.asarray(x, dtype=np.float32).reshape(B_FULL, C, N)
    wt = np.ascontiguousarray(
        np.asarray(W_qkv, dtype=np.float32)[:MT * 128].T
    ).astype(ml_dtypes.bfloat16)

    in_maps = [
        {"x": np.ascontiguousarray(x5[B_CORE * c:B_CORE * (c + 1)]), "wt": wt}
        for c in range(NCORES)
    ]
    res = run_bass_kernel_spmd(nc, in_maps, core_ids=list(range(NCORES)))
    global LAST_RESULTS
    LAST_RESULTS = res

    rx = np.empty((B_FULL, C, N), np.float32)
    fused_all = np.empty((B_FULL, NN), np.float32)
    for c in range(NCORES):
        out = res.results[c]
        rx[B_CORE * c:B_CORE * (c + 1)] = out["rx"]
        f = out["fus"].reshape(NSB, 2, N, 8, N).transpose(0, 1, 3, 2, 4)
        fused_all[B_CORE * c:B_CORE * (c + 1)] = f.reshape(B_CORE, NN)

    rx = _host_finalize(rx, x5, fused_all)
    return rx.reshape(B_FULL, C, 7, 7)


# revision 18
# speedup vs baseline: 2.1302x; 2.1302x over previous
"""Trainium2 Bass kernel: fused ViT-style attention rollout gating.

Math (per sample b):
  qkT[d, n]   = W_qk[d, :] @ x[b][:, n]          (d = 2*896: q rows then k rows)
  logits[h]   = qT_h.T @ kT_h                    ([49, 49] per head, K = head_dim = 128)
  attn[h]     = softmax(scale * logits[h])       (row-wise; no max-subtraction: |logits*scale| < 2)
  fused       = min_h attn[h]                    ([49, 49])
  rowsum[n]   = sum_m fused[n, m];  colsum[m] = sum_n fused[n, m]
  att[m]      = (colsum[m] + 1) / (49 * (rowsum[m] + 1))
  rx[b]       = x[b] * (1 + att)                 (broadcast over channels)

The reference's flat-topk masking quirk only touches global sample 0; it is
corrected exactly on the host from the device-exported `fused` matrices.

Sharding: pure data-parallel, 128 samples per core across 8 cores.
Layout per core/sub-batch (SB=16 samples): x is loaded as 7 c-tiles
[128c, 16b, 49n]; attention tiles pack 16 samples as 2 vertical partition
blocks (base 0 / 64, via PE column tiling) x 8 horizontal 49-col slots.
"""

import numpy as np
import ml_dtypes

# ---- problem constants (hardcoded per contest rules) ----
B_FULL = 1024
C = 896
N = 49                   # tokens (7x7)
NH = 7                   # heads
HD = 128                 # head dim
NCORES = 8
B_CORE = B_FULL // NCORES   # 128
SB = 16                     # samples per sub-batch
NSB = B_CORE // SB          # 8 sub-batches
CT = C // 128               # 7 contraction tiles
MT = 2 * C // 128           # 14 output d-tiles (q then k)
HF = 8 * N                  # 392 = half free width (8 horizontal samples)
FDX = SB * N                # 784
SCALE = float(HD) ** -0.5
NN = N * N                  # 2401
KEEP = NN - int(NN * 0.9)   # 241 largest kept out of topk(smallest 90%)

_CACHE = {}
LAST_RESULTS = None  # BassKernelResults of the most recent kernel() call


def _build(nsb=NSB):
    import concourse.tile as tile
    from concourse import bacc, mybir

    dt = mybir.dt
    f32 = dt.float32
    bf16 = dt.bfloat16
    AF = mybir.ActivationFunctionType
    ALU = mybir.AluOpType
    AX = mybir.AxisListType

    nc = bacc.Bacc("TRN2", target_bir_lowering=False, debug=False,
                   num_devices=NCORES)
    # x and rx travel in channel-major layout [C, B, N] (host transposes both
    # ways) so every DMA run is 16*49*4 = 3136 contiguous bytes
    x_d = nc.dram_tensor("x", [C, B_CORE, N], f32, kind="ExternalInput").ap()
    wt_d = nc.dram_tensor("wt", [C, MT * 128], bf16, kind="ExternalInput").ap()
    rx_d = nc.dram_tensor("rx", [C, B_CORE, N], f32, kind="ExternalOutput").ap()
    fus_d = nc.dram_tensor("fus", [NSB, 2, N, HF], f32,
                           kind="ExternalOutput").ap()

    with tile.TileContext(nc) as tc:
        with (
            tc.tile_pool(name="w", bufs=1) as wpool,
            tc.tile_pool(name="xt", bufs=2) as xtpool,
            tc.tile_pool(name="xb", bufs=2) as xbpool,
            tc.tile_pool(name="qk", bufs=2) as qkpool,
            tc.tile_pool(name="e", bufs=2) as epool,
            tc.tile_pool(name="sm", bufs=2) as spool,
            tc.tile_pool(name="qps", bufs=2, space="PSUM") as qpspool,
            tc.tile_pool(name="aps", bufs=2, space="PSUM") as apspool,
            tc.tile_pool(name="cps", bufs=2, space="PSUM") as cpspool,
            tc.tile_pool(name="dram", bufs=2, space="DRAM") as dpool,
        ):
            # ---- one-time: weights (pre-transposed on host) + colsum ones ----
            wtb = []
            for k in range(CT):
                w = wpool.tile([128, MT * 128], bf16, tag=f"w{k}")
                nc.sync.dma_start(out=w[:], in_=wt_d[128 * k:128 * (k + 1), :])
                wtb.append(w)
            ones2 = wpool.tile([128, 2], f32, tag="ones2")
            nc.vector.memset(ones2[:], 0.0)
            nc.vector.memset(ones2[0:N, 0:1], 1.0)
            nc.vector.memset(ones2[64:64 + N, 1:2], 1.0)

            for s in range(nsb):
                # ---- load x c-tiles + cast to bf16 ----
                xts, xbs = [], []
                for k in range(CT):
                    xt = xtpool.tile([128, SB, N], f32, tag=f"xt{k}")
                    src = x_d[128 * k:128 * (k + 1),
                              SB * s:SB * (s + 1), :]
                    nc.sync.dma_start(out=xt[:], in_=src)
                    xb = xbpool.tile([128, FDX], bf16, tag=f"xb{k}")
                    nc.vector.tensor_copy(
                        out=xb[:], in_=xt[:].rearrange("c b n -> c (b n)"))
                    xts.append(xt)
                    xbs.append(xb)

                # ---- qkv projection: qkT[m] = W[m-tile] @ x ----
                # qk tiles carry 16 zero columns of tail padding so the
                # M=64-wide attention stationary reads below never run off
                # the end (jj=15 reads columns 735:799)
                qks = []
                for m in range(MT):
                    qk = qkpool.tile([128, FDX + 16], bf16, tag=f"qk{m}")
                    nc.vector.memset(qk[:, FDX:], 0.0)
                    for half in range(2):
                        q = qpspool.tile([128, HF], f32, tag=f"qps{half}")
                        for k in range(CT):
                            nc.tensor.matmul(
                                q[:],
                                lhsT=wtb[k][:, 128 * m:128 * (m + 1)],
                                rhs=xbs[k][:, HF * half:HF * (half + 1)],
                                start=(k == 0), stop=(k == CT - 1))
                        nc.scalar.copy(
                            out=qk[:, HF * half:HF * (half + 1)], in_=q[:])
                    qks.append(qk)

                # ---- attention logits + exp + segmented row sums ----
                S = spool.tile([128, NH, 8], f32, tag="S")
                Es = []
                for h in range(NH):
                    A = apspool.tile([128, HF], f32, tag="A")
                    for j in range(8):
                        # vertical block 0: samples j     -> partitions 0:49
                        # vertical block 1: samples 8 + j -> partitions 64:113
                        # (M=64 stationary: rows 49:64 / 113:128 get junk dot
                        # products from neighboring columns — finite, masked
                        # out downstream — so every PSUM row is written)
                        nc.tensor.matmul(
                            A[0:64, N * j:N * (j + 1)],
                            lhsT=qks[h][:, N * j:N * j + 64],
                            rhs=qks[NH + h][:, N * j:N * (j + 1)],
                            start=True, stop=True)
                        nc.tensor.matmul(
                            A[64:128, N * j:N * (j + 1)],
                            lhsT=qks[h][:, N * (8 + j):N * (8 + j) + 64],
                            rhs=qks[NH + h][:, N * (8 + j):N * (9 + j)],
                            start=True, stop=True)
                    E = epool.tile([128, 8, N], f32, tag=f"E{h}")
                    nc.scalar.activation(
                        out=E[:], in_=A[:].rearrange("p (j n) -> p j n", n=N),
                        func=AF.Exp, scale=SCALE)
                    nc.vector.reduce_sum(out=S[:, h, :], in_=E[:], axis=AX.X)
                    Es.append(E)

                # ---- normalize + min over heads ----
                R = spool.tile([128, NH, 8], f32, tag="R")
                nc.vector.reciprocal(out=R[:], in_=S[:])
                F = spool.tile([128, 8, N], f32, tag="F")
                T = spool.tile([128, 8, N], f32, tag="T")
                for h in range(NH):
                    rb = R[:, h, :].unsqueeze(2).broadcast_to([128, 8, N])
                    dst = F if h == 0 else T
                    nc.vector.tensor_tensor(
                        out=dst[:], in0=Es[h][:], in1=rb, op=ALU.mult)
                    if h > 0:
                        nc.vector.tensor_tensor(
                            out=F[:], in0=F[:], in1=T[:], op=ALU.min)

                # export fused for the host-side topk mask correction
                nc.gpsimd.dma_start(
                    out=fus_d[s, 0],
                    in_=F[0:N].rearrange("p j n -> p (j n)"))
                nc.gpsimd.dma_start(
                    out=fus_d[s, 1],
                    in_=F[64:64 + N].rearrange("p j n -> p (j n)"))

                # ---- rowsum (free-dim reduce) and colsum (PE ones-matmul) ----
                RS = spool.tile([128, 8], f32, tag="RS")
                nc.vector.reduce_sum(out=RS[:], in_=F[:], axis=AX.X)
                Cp = cpspool.tile([2, HF], f32, tag="C")
                nc.tensor.matmul(
                    Cp[:], lhsT=ones2[:],
                    rhs=F[:].rearrange("p j n -> p (j n)"),
                    start=True, stop=True)

                # ---- reshuffle both to [16 samples, 49] via DRAM bounce ----
                rs_dram = dpool.tile([2, 8, N], f32, tag="rsd")
                nc.gpsimd.dma_start(
                    out=rs_dram[0].transpose([1, 0]), in_=RS[0:N, :])
                nc.gpsimd.dma_start(
                    out=rs_dram[1].transpose([1, 0]), in_=RS[64:64 + N, :])
                Rs = spool.tile([SB, N], f32, tag="Rs")
                nc.gpsimd.dma_start(
                    out=Rs[:],
                    in_=rs_dram[:].rearrange("k j n -> (k j) n"))

                Csb = spool.tile([2, 8, N], f32, tag="Csb")
                nc.scalar.copy(
                    out=Csb[:], in_=Cp[:].rearrange("p (j n) -> p j n", n=N))
                cs_dram = dpool.tile([2, 8, N], f32, tag="csd")
                nc.gpsimd.dma_start(out=cs_dram[:], in_=Csb[:])
                Cs = spool.tile([SB, N], f32, tag="Cs")
                nc.gpsimd.dma_start(
                    out=Cs[:],
                    in_=cs_dram[:].rearrange("k j n -> (k j) n"))

                # ---- att + 1 = (colsum+1)/(49*(rowsum+1)) + 1 ----
                D = spool.tile([SB, N], f32, tag="D")
                nc.scalar.activation(out=D[:], in_=Rs[:], func=AF.Copy,
                                     scale=float(N), bias=float(N))
                nc.vector.reciprocal(out=D[:], in_=D[:])
                M1 = spool.tile([SB, N], f32, tag="M1")
                nc.vector.tensor_scalar_add(M1[:], Cs[:], 1.0)
                nc.vector.tensor_tensor(
                    out=M1[:], in0=M1[:], in1=D[:], op=ALU.mult)
                nc.vector.tensor_scalar_add(M1[:], M1[:], 1.0)

                # broadcast multiplier to all 128 partitions via DRAM
                m1_dram = dpool.tile([FDX], f32, tag="m1d")
                nc.gpsimd.dma_start(out=m1_dram[:], in_=M1[:])
                M1b = spool.tile([128, SB, N], f32, tag="M1b")
                nc.gpsimd.dma_start(
                    out=M1b[:],
                    in_=m1_dram[:].rearrange("(b n) -> b n",
                                             n=N).partition_broadcast(128))

                # ---- rx = x * (1 + att), in place, then store ----
                # multiply on GpSimd (otherwise idle), store via the
                # Activation HWDGE queue so loads/stores issue in parallel
                for k in range(CT):
                    nc.gpsimd.tensor_tensor(
                        out=xts[k][:], in0=xts[k][:], in1=M1b[:], op=ALU.mult)
                    dst = rx_d[128 * k:128 * (k + 1),
                               SB * s:SB * (s + 1), :]
                    nc.scalar.dma_start(out=dst, in_=xts[k][:])

    nc.compile()
    return nc


def _get_program(nsb=NSB):
    if nsb not in _CACHE:
        _CACHE[nsb] = _build(nsb)
    return _CACHE[nsb]


def _host_finalize(rx, x5, fused_all):
    """Exact replication of the reference's flat-topk masking quirk.

    Only global sample 0 is affected: its fused matrix is masked by the
    union of all samples' bottom-90% index sets (computed from the
    device-exported fused matrices), then its att row is rebuilt exactly.
    """
    thr = np.partition(fused_all, NN - KEEP, axis=1)[:, NN - KEEP]
    in_top = fused_all >= thr[:, None]
    zero_mask = (~in_top).any(axis=0)
    zero_mask[0] = False
    f0 = fused_all[0].copy()
    f0[zero_mask] = 0.0
    fm = f0.reshape(N, N)
    rowsum = fm.sum(axis=1)
    colsum = fm.sum(axis=0)
    att0 = (colsum + 1.0) / (N * (rowsum + 1.0))
    rx[0] = x5[0] * (1.0 + att0[None, :].astype(np.float32))
    return rx


def _par(fn, n):
    from concurrent.futures import ThreadPoolExecutor
    with ThreadPoolExecutor(max_workers=n) as ex:
        list(ex.map(fn, range(n)))


def kernel(x, W_qkv):
    from concourse.bass_utils import run_bass_kernel_spmd

    nc = _get_program()
    x5 = np.asarray(x, dtype=np.float32).reshape(B_FULL, C, N)
    wt = np.ascontiguousarray(
        np.asarray(W_qkv, dtype=np.float32)[:MT * 128].T
    ).astype(ml_dtypes.bfloat16)

    # per-core channel-major shards [C, B_CORE, N]
    shards = [np.empty((C, B_CORE, N), np.float32) for _ in range(NCORES)]
    _par(lambda c: np.copyto(
        shards[c], x5[B_CORE * c:B_CORE * (c + 1)].transpose(1, 0, 2)), NCORES)

    in_maps = [{"x": shards[c], "wt": wt} for c in range(NCORES)]
    res = run_bass_kernel_spmd(nc, in_maps, core_ids=list(range(NCORES)))
    global LAST_RESULTS
    LAST_RESULTS = res

    rx = np.empty((B_FULL, C, N), np.float32)
    fused_all = np.empty((B_FULL, NN), np.float32)

    def _gather(c):
        out = res.results[c]
        rx[B_CORE * c:B_CORE * (c + 1)] = out["rx"].transpose(1, 0, 2)
        f = out["fus"].reshape(NSB, 2, N, 8, N).transpose(0, 1, 3, 2, 4)
        fused_all[B_CORE * c:B_CORE * (c + 1)] = f.reshape(B_CORE, NN)

    _par(_gather, NCORES)

    rx = _host_finalize(rx, x5, fused_all)
    return rx.reshape(B_FULL, C, 7, 7)
